# revision 1
# baseline (speedup 1.0000x reference)
"""Multi-head attention (b=8, n=1024, dim=1024, heads=16) on 8 Trainium2 cores.

Strategy: pure data-parallel over batch — core b computes attention for x[b].
No collectives. Weights are replicated to every core.

Per-core pipeline (all matmuls in fp32r = full-rate fp32 PE mode):
  1. host feeds x[b].T so the contraction dim (model dim) lands on SBUF
     partitions with no on-device transpose.
  2. v = x @ Wv computed in natural [n, d] orientation, stored with an
     appended ones-column per head (v_aug) so that the attention row-sums
     fall out of the AV matmul for free.
  3. qT/kT = (x @ Wq/Wk)^T computed directly in transposed orientation
     (head-dim on partitions) — exactly what the S^T matmul wants. Emitted
     interleaved with per-head-pair attention so PE work stays dense.
  4. per head: S^T[j,i] = k_j . q_i via K=64 matmuls (head pairs alternate
     PE row groups 0-63/64-127 so consecutive matmuls can overlap);
     P^T = exp(SCALE*S^T) on the scalar engine (softmax max-subtraction is
     skipped: |SCALE*S| < ~2 for these magnitudes, exp is exact to 2 ulp);
     O^T_aug = V_aug^T @ P^T accumulated over j. Row 64 of O^T_aug is the
     softmax denominator. O^T_aug is staged to SBUF (freeing PSUM fast)
     and normalized lazily: legacy DVE reciprocal (the approx variants are
     broken on this HW), partition-broadcast via a DRAM bounce, multiply.
  5. out = O_cat @ Wout + b_out with O^T tiles as the stationary operand,
     bias added on the vector engine from a broadcast SBUF copy.
"""

import numpy as np

N = 1024
D = 1024
H = 16
DH = 64
P = 128
SCALE = float(D) ** (-0.5)
NCORES = 8

_STATE: dict = {}


def _emit(tc, xT, wqkv, wout, bout, out, mm_dt):
    import concourse.mybir as mybir

    nc = tc.nc
    f32 = mybir.dt.float32
    EXP = mybir.ActivationFunctionType.Exp

    from contextlib import ExitStack

    with ExitStack() as ctx:
        persist = ctx.enter_context(tc.tile_pool(name="persist", bufs=1))
        v_aug = [persist.tile([P, H * 65], mm_dt, tag=f"vaug{nt}", name=f"vaug{nt}") for nt in range(8)]
        oT = [persist.tile([P, N], mm_dt, tag=f"oT{hp}", name=f"oT{hp}") for hp in range(8)]
        bias_sb = persist.tile([P, N], f32, tag="bias", name="bias_sb")
        nc.sync.dma_start(bias_sb[:], bout[0:1, :].broadcast_to([P, N]))
        ones_f = persist.tile([P, H], f32, tag="ones", name="ones_f")
        nc.vector.memset(ones_f[:], 1.0)

        with tc.tile_pool(name="xt", bufs=1) as xt_pool, \
             tc.tile_pool(name="wstream", bufs=2) as wpool, \
             tc.tile_pool(name="qk", bufs=5) as qk_pool, \
             tc.tile_pool(name="p", bufs=3) as p_pool, \
             tc.tile_pool(name="stg", bufs=2) as stg_pool, \
             tc.tile_pool(name="norm", bufs=1) as n_pool, \
             tc.tile_pool(name="dbounce", bufs=2, space="DRAM") as d_pool, \
             tc.tile_pool(name="sps", bufs=1, space="PSUM") as sp, \
             tc.tile_pool(name="otps", bufs=1, space="PSUM") as op:

            xT_sb = []
            for k in range(8):
                t = xt_pool.tile([P, N], mm_dt, tag=f"xt{k}", name=f"xt{k}")
                nc.sync.dma_start(t[:], xT[k * P:(k + 1) * P, :])
                xT_sb.append(t)

            # ones column of v_aug via DVE (f32 -> f32r cast copy)
            for nt in range(8):
                nc.vector.tensor_copy(
                    v_aug[nt][:].rearrange("p (h e) -> p h e", e=65)[:, :, 64:65],
                    ones_f[:, :, None])

            def load_w(cols, lo, hi):
                tiles = []
                for k in range(8):
                    w = wpool.tile([P, 512], mm_dt, tag=f"w{k}", name=f"w{k}")
                    nc.sync.dma_start(w[:], wqkv[k * P:(k + 1) * P, lo:hi])
                    tiles.append(w)
                return tiles

            # ---- V projection (natural layout + strided copy into v_aug) ----
            for ic in range(2):
                wv = load_w(512, 2048 + ic * 512, 2048 + (ic + 1) * 512)
                for nt in range(8):
                    ps = sp.tile([P, N], f32, tag=f"s{nt % 2}", name="v_ps")
                    for k in range(8):
                        nc.tensor.matmul(
                            ps[:, 0:512], lhsT=xT_sb[k][:, nt * P:(nt + 1) * P],
                            rhs=wv[k][:], start=(k == 0), stop=(k == 7))
                    dst = v_aug[nt][:].rearrange("p (h e) -> p h e", e=65)[:, 8 * ic:8 * ic + 8, 0:64]
                    nc.vector.tensor_copy(dst, ps[:, 0:512].rearrange("p (h e) -> p h e", e=64))

            # ---- interleaved qT/kT projection + attention, per head pair ----
            def project(wt_chunk, off, stag):
                """One [128, n] tile of qkv^T (128 rows of q or k)."""
                t = qk_pool.tile([P, N], mm_dt, tag="qk", name="qk_t")
                ps = sp.tile([P, N], f32, tag=stag, name="proj_ps")
                for ic in range(2):
                    for k in range(8):
                        nc.tensor.matmul(
                            ps[:, ic * 512:(ic + 1) * 512],
                            lhsT=wt_chunk[k][:, off:off + P],
                            rhs=xT_sb[k][:, ic * 512:(ic + 1) * 512],
                            start=(k == 0), stop=(k == 7))
                nc.vector.tensor_copy(t[:], ps[:])
                return t

            def emit_norm(hpair, stg):
                """Deferred softmax normalization for a staged head pair."""
                for half in range(2):
                    bp = half * 64
                    rcp = n_pool.tile([65, N], f32, tag="rcp", name="rcp")
                    for ic in range(2):
                        sl = slice(ic * 512, (ic + 1) * 512)
                        nc.vector.reciprocal(rcp[64:65, sl], stg[half][ic][64:65, :])
                    rcp_d = d_pool.tile([1, N], f32, tag="rcp_d", name="rcp_d")
                    nc.gpsimd.dma_start(rcp_d[:], rcp[64:65, :])
                    bc = n_pool.tile([64, N], f32, tag="bc", name="bc")
                    nc.gpsimd.dma_start(bc[:], rcp_d[0:1, :].broadcast_to([64, N]))
                    for ic in range(2):
                        nc.vector.tensor_mul(
                            oT[hpair][bp:bp + 64, ic * 512:(ic + 1) * 512],
                            stg[half][ic][0:64, :], bc[0:64, ic * 512:(ic + 1) * 512])

            pending = None
            for grp in range(2):
                wq_c = load_w(512, grp * 512, (grp + 1) * 512)
                wk_c = load_w(512, 1024 + grp * 512, 1024 + (grp + 1) * 512)
                for sub in range(4):
                    hpair = grp * 4 + sub
                    off = sub * P
                    qTt = project(wq_c, off, "s0")
                    kTt = project(wk_c, off, "s1")
                    if pending is not None:
                        emit_norm(*pending)
                        pending = None

                    ot_ps = [[op.tile([65, 512], f32, tag=f"ot{half}{ic}", name=f"ot_ps{half}{ic}")
                              for ic in range(2)] for half in range(2)]
                    for jt in range(8):
                        sps_t = [sp.tile([P, N], f32, tag=f"s{half}", name=f"s_ps{half}")
                                 for half in range(2)]
                        # both heads' S^T; alternating base-partition 0/64 gives
                        # the PE disjoint row groups to overlap
                        for ic in range(2):
                            for half in range(2):
                                bp = half * 64
                                nc.tensor.matmul(
                                    sps_t[half][:, ic * 512:(ic + 1) * 512],
                                    lhsT=kTt[bp:bp + 64, jt * P:(jt + 1) * P],
                                    rhs=qTt[bp:bp + 64, ic * 512:(ic + 1) * 512],
                                    start=True, stop=True)
                        for half in range(2):
                            h = 2 * hpair + half
                            p_sb = p_pool.tile([P, N], mm_dt, tag="p", name="p_sb")
                            nc.scalar.activation(p_sb[:], sps_t[half][:], EXP, scale=SCALE)
                            va = v_aug[jt][:].rearrange("p (h e) -> p h e", e=65)[:, h, :]
                            for ic in range(2):
                                nc.tensor.matmul(
                                    ot_ps[half][ic][:], lhsT=va,
                                    rhs=p_sb[:, ic * 512:(ic + 1) * 512],
                                    start=(jt == 0), stop=(jt == 7))
                    # stage unnormalized O^T (+ row-sum row) to SBUF so the
                    # PSUM accumulators free immediately; normalization is
                    # deferred to after the next pair's projections (keeps the
                    # PE dense; the DVE reciprocal is slow and off-path here).
                    stg = []
                    for half in range(2):
                        row = []
                        for ic in range(2):
                            s_t = stg_pool.tile([65, 512], f32, tag=f"stg{half}{ic}",
                                                name=f"stg{half}{ic}")
                            nc.vector.tensor_copy(s_t[:], ot_ps[half][ic][:])
                            row.append(s_t)
                        stg.append(row)
                    pending = (hpair, stg)
            emit_norm(*pending)

        # ---------------- output projection ----------------
        with tc.tile_pool(name="wo", bufs=2) as wo_pool, \
             tc.tile_pool(name="osb", bufs=4) as o_pool, \
             tc.tile_pool(name="fps", bufs=4, space="PSUM") as fp:
            for ch in range(2):
                wo = []
                for hp in range(8):
                    w = wo_pool.tile([P, 512], mm_dt, tag=f"wo{hp}", name=f"wo{hp}")
                    nc.sync.dma_start(w[:], wout[hp * P:(hp + 1) * P, ch * 512:(ch + 1) * 512])
                    wo.append(w)
                for it in range(8):
                    ps = fp.tile([P, 512], f32, tag="f", name="f_ps")
                    for hp in range(8):
                        nc.tensor.matmul(
                            ps[:], lhsT=oT[hp][:, it * P:(it + 1) * P],
                            rhs=wo[hp][:], start=(hp == 0), stop=(hp == 7))
                    osb = o_pool.tile([P, 512], f32, tag="o", name="o_sb")
                    nc.vector.tensor_add(osb[:], ps[:], bias_sb[:, ch * 512:(ch + 1) * 512])
                    nc.sync.dma_start(out[it * P:(it + 1) * P, ch * 512:(ch + 1) * 512], osb[:])


def build(mm_dtype: str = "float32r"):
    """Build + compile the Bass program once per process."""
    key = ("nc", mm_dtype)
    if key in _STATE:
        return _STATE[key]
    import concourse.mybir as mybir
    import concourse.tile as tile
    from concourse import bacc

    nc = bacc.Bacc("TRN2", target_bir_lowering=False, debug=False,
                   enable_asserts=False, num_devices=NCORES)
    f32 = mybir.dt.float32
    mm_dt = getattr(mybir.dt, mm_dtype)
    xT = nc.dram_tensor("xT", [D, N], mm_dt, kind="ExternalInput").ap()
    wqkv = nc.dram_tensor("wqkv", [D, 3 * D], mm_dt, kind="ExternalInput").ap()
    wout = nc.dram_tensor("wout", [D, D], mm_dt, kind="ExternalInput").ap()
    bout = nc.dram_tensor("bout", [1, D], f32, kind="ExternalInput").ap()
    out = nc.dram_tensor("out", [N, D], f32, kind="ExternalOutput").ap()

    with tile.TileContext(nc) as tc:
        _emit(tc, xT, wqkv, wout, bout, out, mm_dt=mm_dt)
    nc.compile()
    _STATE[key] = nc
    return nc


def make_in_maps(x, w_qkv, w_out, b_out):
    x = np.ascontiguousarray(np.asarray(x, np.float32))
    w_qkv = np.ascontiguousarray(np.asarray(w_qkv, np.float32))
    w_out = np.ascontiguousarray(np.asarray(w_out, np.float32))
    b_out = np.ascontiguousarray(np.asarray(b_out, np.float32)).reshape(1, D)
    return [
        {"xT": np.ascontiguousarray(x[b].T), "wqkv": w_qkv, "wout": w_out, "bout": b_out}
        for b in range(x.shape[0])
    ]


def run(x, w_qkv, w_out, b_out, trace=False, mm_dtype="float32r"):
    from concourse.bass_utils import run_bass_kernel_spmd

    nc = build(mm_dtype)
    in_maps = make_in_maps(x, w_qkv, w_out, b_out)
    res = run_bass_kernel_spmd(nc, in_maps, core_ids=list(range(NCORES)), trace=trace)
    outs = np.stack([res.results[c]["out"] for c in range(NCORES)])
    return outs, res


def kernel(x, w_qkv, w_out, b_out):
    outs, _ = run(x, w_qkv, w_out, b_out, trace=False)
    return outs.astype(np.float32)



# revision 2
# speedup vs baseline: 1.7658x; 1.7658x over previous
"""Multi-head attention (b=8, n=1024, dim=1024, heads=16) on 8 Trainium2 cores.

Strategy: pure data-parallel over batch — core b computes attention for x[b].
No collectives. Weights are replicated to every core.

Per-core pipeline (all matmuls in bf16 — rel tolerance 2e-2 leaves plenty of
room, and bf16 keeps the PE's HAM clock-gate at 8/8 by letting every stage
double-buffer in PSUM):

  1. host feeds x[b].T, w_qkv, w_out pre-cast to bf16; the contraction dim
     (model dim) lands on SBUF partitions with no on-device transpose.
  2. v = x @ Wv in natural [n, d] layout, stored with an appended ones-column
     per head (v_aug) so attention row-sums fall out of the AV matmul free.
  3. qT/kT = (x @ Wq/Wk)^T computed directly transposed (head-dim on
     partitions) per head-pair; the 32 projection matmuls of the NEXT pair
     are drip-fed into the current pair's attention loop so the PE queue
     always has runnable work while exp() results are pending.
  4. per head, per j-tile: S^T = k_j . q_i (two 512-wide matmuls into a
     rotating 2-deep PSUM tile); P^T = exp(SCALE*S^T) on the scalar engine
     (max-subtraction skipped: |SCALE*S| < ~2.5); AV accumulates
     O^T_aug = V_aug^T @ P^T over j into a persistent [65, 1024] PSUM tile.
     The AV for tile j is emitted one slot late so it consumes an exp()
     that is already finished — the PE never blocks on the scalar engine.
  5. softmax normalization: the denominator row of O^T_aug is reshaped via a
     tiny DRAM bounce to [128, 8], hit with one full-width DVE reciprocal
     (~100ns, vs 3.3us for the 1-partition layout), broadcast back along
     partitions through DRAM, and multiplied in as bf16.
  6. out = O_cat @ Wout + b_out with O^T tiles stationary, bias on the DVE.

PSUM budget (8 banks): S rotation 2x[128,1024]f32 = 4, AV accumulator
[65,1024]f32 = 2, projection accumulator [128,1024]f32 = 2.
"""

import numpy as np

N = 1024
D = 1024
H = 16
DH = 64
P = 128
SCALE = float(D) ** (-0.5)
NCORES = 8

_STATE: dict = {}


def _emit(tc, xT, wqkv, wout, bout, out):
    import concourse.mybir as mybir
    from collections import deque
    from contextlib import ExitStack

    nc = tc.nc
    f32 = mybir.dt.float32
    bf16 = mybir.dt.bfloat16
    EXP = mybir.ActivationFunctionType.Exp

    with ExitStack() as ctx:
        persist = ctx.enter_context(tc.tile_pool(name="persist", bufs=1))
        xT_sb = [persist.tile([P, N], bf16, tag=f"xt{k}", name=f"xt{k}") for k in range(8)]
        w_sb = [persist.tile([P, 3 * D], bf16, tag=f"w{k}", name=f"w{k}") for k in range(8)]
        wo_sb = [persist.tile([P, D], bf16, tag=f"wo{p}", name=f"wo{p}") for p in range(8)]
        v_aug = [persist.tile([P, H * 65], bf16, tag=f"va{nt}", name=f"va{nt}") for nt in range(8)]
        oT_sb = [persist.tile([P, N], bf16, tag=f"oT{p}", name=f"oT{p}") for p in range(8)]
        qT_sb = [persist.tile([P, N], bf16, tag=f"qT{i}", name=f"qT{i}") for i in range(2)]
        kT_sb = [persist.tile([P, N], bf16, tag=f"kT{i}", name=f"kT{i}") for i in range(2)]
        bias_sb = persist.tile([P, D], f32, tag="bias", name="bias")

        # input DMAs: x first, then the V columns of w_qkv (needed first),
        # then Q/K columns, then the output-projection weights.
        for k in range(8):
            nc.sync.dma_start(xT_sb[k][:], xT[k * P:(k + 1) * P, :])
        for k in range(8):
            nc.sync.dma_start(w_sb[k][:, 2048:3072], wqkv[k * P:(k + 1) * P, 2048:3072])
        for k in range(8):
            nc.sync.dma_start(w_sb[k][:, 0:2048], wqkv[k * P:(k + 1) * P, 0:2048])
        nc.sync.dma_start(bias_sb[:], bout[0:1, :].broadcast_to([P, D]))
        for p in range(8):
            nc.sync.dma_start(wo_sb[p][:], wout[p * P:(p + 1) * P, :])

        # ones columns of v_aug
        for nt in range(8):
            nc.vector.memset(
                v_aug[nt][:].rearrange("p (h e) -> p h e", e=65)[:, :, 64:65], 1.0)

        with tc.tile_pool(name="spool", bufs=2, space="PSUM") as sp, \
             tc.tile_pool(name="pjpool", bufs=1, space="PSUM") as pjp, \
             tc.tile_pool(name="avpool", bufs=1, space="PSUM") as avp, \
             tc.tile_pool(name="ppool", bufs=3) as pp, \
             tc.tile_pool(name="stgpool", bufs=2) as stgp, \
             tc.tile_pool(name="npool", bufs=2) as npl, \
             tc.tile_pool(name="bcpool", bufs=2) as bcp, \
             tc.tile_pool(name="dpool", bufs=4, space="DRAM") as dp:

            # ---- V projection (natural layout, strided drain into v_aug) ----
            for nt in range(8):
                ps = sp.tile([P, N], f32, tag="s", name="v_ps")
                for ic in range(2):
                    for k in range(8):
                        nc.tensor.matmul(
                            ps[:, ic * 512:(ic + 1) * 512],
                            lhsT=xT_sb[k][:, nt * P:(nt + 1) * P],
                            rhs=w_sb[k][:, 2048 + ic * 512:2048 + (ic + 1) * 512],
                            start=(k == 0), stop=(k == 7))
                nc.vector.tensor_copy(
                    v_aug[nt][:].rearrange("p (h e) -> p h e", e=65)[:, :, 0:64],
                    ps[:].rearrange("p (h e) -> p h e", e=64))

            # ---- q/k projection step lists (drip-fed into attention) ----
            def proj_steps(pair):
                steps = []
                for dst, base in ((qT_sb[pair % 2], pair * P),
                                  (kT_sb[pair % 2], D + pair * P)):
                    holder = {}

                    def mk_mm(ic, k, base=base, holder=holder):
                        def go():
                            if "t" not in holder:
                                holder["t"] = pjp.tile([P, N], f32, tag="pj", name="pj_ps")
                            nc.tensor.matmul(
                                holder["t"][:, ic * 512:(ic + 1) * 512],
                                lhsT=w_sb[k][:, base:base + P],
                                rhs=xT_sb[k][:, ic * 512:(ic + 1) * 512],
                                start=(k == 0), stop=(k == 7))
                        return go

                    for ic in range(2):
                        for k in range(8):
                            steps.append(mk_mm(ic, k))

                    def mk_drain(dst=dst, holder=holder):
                        def go():
                            nc.vector.tensor_copy(dst[:], holder["t"][:])
                        return go

                    steps.append(mk_drain())
                return steps

            # prime pair 0 before the attention loop
            for st in proj_steps(0):
                st()

            av_holder = {}

            def emit_head_tail(h, av_t):
                pair, bp = h // 2, (h % 2) * 64
                stg = stgp.tile([65, N], bf16, tag="stg", name="stg")
                nc.vector.tensor_copy(stg[:], av_t[:])
                # denominator -> [128, 8] via DRAM bounce, full-width recip,
                # back to a [64, N] partition-broadcast, multiply in.
                d_den = dp.tile([1, N], bf16, tag="dden", name="d_den")
                nc.gpsimd.dma_start(d_den[:], stg[64:65, :])
                den = npl.tile([P, 8], bf16, tag="den", name="den")
                nc.gpsimd.dma_start(
                    den[:], d_den[0:1, :].rearrange("o (p c) -> (o p) c", p=P))
                rcp = npl.tile([P, 8], f32, tag="rcp", name="rcp")
                nc.vector.reciprocal(rcp[:], den[:])
                rcpb = npl.tile([P, 8], bf16, tag="rcpb", name="rcpb")
                nc.vector.tensor_copy(rcpb[:], rcp[:])
                d_rcp = dp.tile([1, N], bf16, tag="drcp", name="d_rcp")
                nc.gpsimd.dma_start(
                    d_rcp[0:1, :].rearrange("o (p c) -> (o p) c", p=P), rcpb[:])
                bc = bcp.tile([64, N], bf16, tag="bc", name="bc")
                nc.gpsimd.dma_start(bc[:], d_rcp[0:1, :].broadcast_to([64, N]))
                nc.vector.tensor_mul(oT_sb[pair][bp:bp + 64, :], stg[0:64, :], bc[:])

            def emit_av(h, p_t, jt):
                if h not in av_holder:
                    av_holder[h] = avp.tile([65, N], f32, tag="av", name="av_ps")
                t = av_holder[h]
                for ic in range(2):
                    nc.tensor.matmul(
                        t[0:65, ic * 512:(ic + 1) * 512],
                        lhsT=v_aug[jt][:, h * 65:(h + 1) * 65],
                        rhs=p_t[:, ic * 512:(ic + 1) * 512],
                        start=(jt == 0), stop=(jt == 7))
                if jt == 7:
                    emit_head_tail(h, t)
                    del av_holder[h]

            proj_q = deque()
            pend_av = None
            for h in range(H):
                pair, bp = h // 2, (h % 2) * 64
                if h % 2 == 0 and pair < 7:
                    proj_q.extend(proj_steps(pair + 1))
                for jt in range(8):
                    s_t = sp.tile([P, N], f32, tag="s", name="s_ps")
                    for ic in range(2):
                        nc.tensor.matmul(
                            s_t[:, ic * 512:(ic + 1) * 512],
                            lhsT=kT_sb[pair % 2][bp:bp + 64, jt * P:(jt + 1) * P],
                            rhs=qT_sb[pair % 2][bp:bp + 64, ic * 512:(ic + 1) * 512],
                            start=True, stop=True)
                    p_t = pp.tile([P, N], bf16, tag="p", name="p_sb")
                    nc.scalar.activation(p_t[:], s_t[:], EXP, scale=SCALE)
                    if pend_av is not None:
                        emit_av(*pend_av)
                    pend_av = (h, p_t, jt)
                    for _ in range(3 if h % 2 == 0 else 2):
                        if proj_q:
                            proj_q.popleft()()
                if h % 2 == 1:
                    while proj_q:
                        proj_q.popleft()()
            emit_av(*pend_av)

        # ---------------- output projection ----------------
        with tc.tile_pool(name="fpool", bufs=4, space="PSUM") as fp, \
             tc.tile_pool(name="opool", bufs=4) as op:
            for ch in range(2):
                for it in range(8):
                    ps = fp.tile([P, 512], f32, tag="f", name="f_ps")
                    for p8 in range(8):
                        nc.tensor.matmul(
                            ps[:], lhsT=oT_sb[p8][:, it * P:(it + 1) * P],
                            rhs=wo_sb[p8][:, ch * 512:(ch + 1) * 512],
                            start=(p8 == 0), stop=(p8 == 7))
                    osb = op.tile([P, 512], f32, tag="o", name="o_sb")
                    nc.vector.tensor_add(osb[:], ps[:], bias_sb[:, ch * 512:(ch + 1) * 512])
                    nc.sync.dma_start(out[it * P:(it + 1) * P, ch * 512:(ch + 1) * 512], osb[:])


def build(mm_dtype: str = "bfloat16"):
    """Build + compile the Bass program once per process."""
    key = ("nc", mm_dtype)
    if key in _STATE:
        return _STATE[key]
    import concourse.mybir as mybir
    import concourse.tile as tile
    from concourse import bacc

    nc = bacc.Bacc("TRN2", target_bir_lowering=False, debug=False,
                   enable_asserts=False, num_devices=NCORES)
    f32 = mybir.dt.float32
    bf16 = mybir.dt.bfloat16
    xT = nc.dram_tensor("xT", [D, N], bf16, kind="ExternalInput").ap()
    wqkv = nc.dram_tensor("wqkv", [D, 3 * D], bf16, kind="ExternalInput").ap()
    wout = nc.dram_tensor("wout", [D, D], bf16, kind="ExternalInput").ap()
    bout = nc.dram_tensor("bout", [1, D], f32, kind="ExternalInput").ap()
    out = nc.dram_tensor("out", [N, D], f32, kind="ExternalOutput").ap()

    with tile.TileContext(nc) as tc:
        _emit(tc, xT, wqkv, wout, bout, out)
    nc.compile()
    _STATE[key] = nc
    return nc


def make_in_maps(x, w_qkv, w_out, b_out):
    import ml_dtypes
    bf16 = ml_dtypes.bfloat16
    x = np.asarray(x, np.float32)
    w_qkv = np.ascontiguousarray(np.asarray(w_qkv, np.float32).astype(bf16))
    w_out = np.ascontiguousarray(np.asarray(w_out, np.float32).astype(bf16))
    b_out = np.ascontiguousarray(np.asarray(b_out, np.float32)).reshape(1, D)
    return [
        {"xT": np.ascontiguousarray(x[b].T.astype(bf16)),
         "wqkv": w_qkv, "wout": w_out, "bout": b_out}
        for b in range(x.shape[0])
    ]


def run(x, w_qkv, w_out, b_out, trace=False, mm_dtype="bfloat16"):
    from concourse.bass_utils import run_bass_kernel_spmd

    nc = build(mm_dtype)
    in_maps = make_in_maps(x, w_qkv, w_out, b_out)
    res = run_bass_kernel_spmd(nc, in_maps, core_ids=list(range(NCORES)), trace=trace)
    outs = np.stack([res.results[c]["out"] for c in range(NCORES)])
    return outs, res


def kernel(x, w_qkv, w_out, b_out):
    outs, _ = run(x, w_qkv, w_out, b_out, trace=False)
    return outs.astype(np.float32)
